# revision 1
# baseline (speedup 1.0000x reference)
"""Causal self-attention (B=2, T=2048, C=1024, NH=16, D=64) on 8 TRN2 NeuronCores.

Sharding: 2-way batch x 4-way head-group tensor parallel (4 heads/core).
Each core computes qkv projection for its 4 heads, causal attention in a
"scores-transposed" layout (k on partitions, q on free dim; softmax without
max-subtraction since |scores| <= ~4), and a c_proj partial product over its
256 hidden channels for all 1024 output features. The host sums the 4
partials per batch (c_proj row-parallel reduction) and concatenates batches.

All matmuls run in bf16 with fp32 PSUM accumulation; softmax denominators and
normalization stay fp32. Host-side prep: shards are transposed/cast so the
device needs no input transposes (contraction dim on partitions).
"""

import numpy as np
import ml_dtypes

import concourse.bass as bass
import concourse.mybir as mybir
import concourse.tile as tile
from concourse import bacc
from concourse.bass_utils import run_bass_kernel_spmd

BF16 = mybir.dt.bfloat16
F32 = mybir.dt.float32

B, T, C = 2, 2048, 1024
NH, D = 16, 64
HPC = NH // 4          # heads per core = 4
CL = HPC * D           # local channels = 256
N_CORES = 8

AF = mybir.ActivationFunctionType


def build_graph():
    nc = bacc.Bacc("TRN2")

    xT_d = nc.declare_dram_parameter("xT", [C, T], BF16, isOutput=False)
    wqk_d = nc.declare_dram_parameter("wqkT", [C, 2 * CL], BF16, isOutput=False)
    wv_d = nc.declare_dram_parameter("wvT", [C, CL], BF16, isOutput=False)
    wp_d = nc.declare_dram_parameter("wpT", [CL, C], BF16, isOutput=False)
    bqk_d = nc.declare_dram_parameter("bqk", [128, 4], F32, isOutput=False)
    bv_d = nc.declare_dram_parameter("bv", [1, CL], BF16, isOutput=False)
    mask_d = nc.declare_dram_parameter("mask", [128, 128], BF16, isOutput=False)
    out_d = nc.declare_dram_parameter("out", [C, T], BF16, isOutput=True)

    NKT = C // 128        # 8 k-tiles over the C contraction
    NTT = T // 128        # 16 t-tiles
    NTC = T // 512        # 4 t-chunks

    with tile.TileContext(nc) as tc:
        with (
            tc.tile_pool(name="persist", bufs=1) as pp,
            tc.tile_pool(name="work", bufs=6) as wp,
            tc.tile_pool(name="bcast", bufs=3) as bcp,
            tc.tile_pool(name="dram", bufs=2, space="DRAM") as dpool,
        ):
            # ---- persistent SBUF tiles + loads (xT / wv first: gate v-phase) ----
            xT_sb = [pp.tile([128, T], BF16, tag=f"xT{i}", name=f"xT{i}") for i in range(NKT)]
            wv_sb = [pp.tile([128, CL], BF16, tag=f"wv{i}", name=f"wv{i}") for i in range(NKT)]
            for i in range(NKT):
                nc.sync.dma_start(xT_sb[i][:], xT_d[128 * i : 128 * (i + 1), :])
                nc.sync.dma_start(wv_sb[i][:], wv_d[128 * i : 128 * (i + 1), :])
            wqk_sb = [pp.tile([128, 2 * CL], BF16, tag=f"wqk{i}", name=f"wqk{i}") for i in range(NKT)]
            for i in range(NKT):
                nc.sync.dma_start(wqk_sb[i][:], wqk_d[128 * i : 128 * (i + 1), :])
            wp_sb = [pp.tile([128, C], BF16, tag=f"wp{i}", name=f"wp{i}") for i in range(CL // 128)]
            for i in range(CL // 128):
                nc.scalar.dma_start(wp_sb[i][:], wp_d[128 * i : 128 * (i + 1), :])
            bqk_sb = pp.tile([128, 4], F32, tag="bqk")
            nc.scalar.dma_start(bqk_sb[:], bqk_d[:])
            bv_sb = pp.tile([1, CL], BF16, tag="bv")
            nc.scalar.dma_start(bv_sb[:], bv_d[:])
            mask_sb = pp.tile([128, 128], BF16, tag="mask")
            nc.scalar.dma_start(mask_sb[:], mask_d[:])
            ones_sb = pp.tile([1, 128], BF16, tag="ones")
            nc.vector.memset(ones_sb[:], 1.0)

            # destination tiles for projections
            qkT_sb = [pp.tile([128, T], BF16, tag=f"qk{i}", name=f"qk{i}") for i in range(4)]
            v_sb = [pp.tile([128, HPC * (D + 1)], BF16, tag=f"v{i}", name=f"v{i}") for i in range(NTT)]
            yT_sb = [pp.tile([128, T], BF16, tag=f"y{i}", name=f"y{i}") for i in range(CL // 128)]

            ps_cm = tc.tile_pool(name="ps", bufs=2, space="PSUM")
            ps = ps_cm.__enter__()
            # ---- v projection, t-major: psum[t128, 4h*64d] = xT_tile^T @ wvT ----
            for tt in range(NTT):
                pvtag = "av" if tt < 8 else "S"
                pv = ps.tile([128, 1024], F32, tag=pvtag, name=f"pv{tt}")[:, 0:CL]
                for kt in range(NKT):
                    nc.tensor.matmul(
                        pv[:],
                        xT_sb[kt][:, 128 * tt : 128 * (tt + 1)],
                        wv_sb[kt][:],
                        start=(kt == 0),
                        stop=False,
                    )
                # add (bV + b_attn_v) via rank-1 outer product: ones[t] x bias[c]
                nc.tensor.matmul(
                    pv[:], ones_sb[:], bv_sb[:], start=False, stop=True
                )
                vt = v_sb[tt][:].rearrange("p (h d) -> p h d", h=HPC)
                nc.vector.tensor_copy(
                    vt[:, :, 0:D], pv[:].rearrange("p (h d) -> p h d", h=HPC)
                )
                nc.vector.memset(vt[:, :, D : D + 1], 1.0)

            # ---- q/k projection, feature-major: psum[f128, t512] ----
            # ft 0/2 (q,k of heads 0-1) first; ft 1/3 deferred into head-0's
            # attention as TensorEngine filler while the Scalar engine is the
            # attention bottleneck (keeps the PE HAM-warm).
            def emit_qk(ft, tcn):
                pq = ps.tile([128, 1024], F32, tag="S", name=f"pq{ft}{tcn}")
                for kt in range(NKT):
                    nc.tensor.matmul(
                        pq[:, 0:512],
                        wqk_sb[kt][:, 128 * ft : 128 * (ft + 1)],
                        xT_sb[kt][:, 512 * tcn : 512 * (tcn + 1)],
                        start=(kt == 0),
                        stop=(kt == NKT - 1),
                    )
                nc.vector.tensor_scalar_add(
                    qkT_sb[ft][:, 512 * tcn : 512 * (tcn + 1)],
                    pq[:, 0:512],
                    bqk_sb[:, ft : ft + 1],
                )

            for ft in (0, 2):
                for tcn in range(NTC):
                    emit_qk(ft, tcn)
            deferred_qk = [(ft, tcn) for ft in (1, 3) for tcn in range(NTC)]

            # ---- attention, per head, two interleaved qc-pair passes ----
            # scoresT[k,q] layout: k on partitions (contraction for av), q free.
            # Softmax skips max-subtraction (|scores| <= ~4). Denominator rides
            # as a 65th "ones" column of v through the av matmul. Each pass
            # covers 2 q-chunks (av accumulator = 2 PSUM banks, double-buffered
            # across passes); the two passes are emitted kt-interleaved so the
            # TensorEngine always has an independent matmul stream while the
            # Scalar engine runs exp (keeps PE busy -> HAM stays warm).
            # Causal masking: the diagonal 128-block of each k-tile is
            # zeroed after exp by a DVE multiply with an upper-triangular
            # 0/1 mask (fully-masked tiles are skipped by loop bounds).
            for h in range(4):
                qT = qkT_sb[h // 2][64 * (h % 2) : 64 * (h % 2) + 64, :]
                kT = qkT_sb[2 + h // 2][64 * (h % 2) : 64 * (h % 2) + 64, :]
                yrow = yT_sb[h // 2][64 * (h % 2) : 64 * (h % 2) + 64, :]
                avs = {}
                steps = [(1, kt) for kt in range(16)] + [(0, kt) for kt in range(8)]
                for si, (p, kt) in enumerate(steps):
                    if h == 0 and si % 3 == 0 and deferred_qk:
                        emit_qk(*deferred_qk.pop(0))
                    pqcs = (0, 1) if p == 0 else (2, 3)
                    if kt == 0:
                        avs[p] = ps.tile(
                            [D + 1, 2, 512], F32, tag="av", name=f"av{h}{p}"
                        )
                    av = avs[p]
                    qc0 = kt // 4
                    so = 128 * kt - 512 * qc0  # diag offset inside chunk qc0
                    qcs = [qc for qc in pqcs if qc >= qc0]
                    S = ps.tile([128, 1024], F32, tag="S", name=f"S{h}{p}{kt}")
                    E = wp.tile([128, 1024], BF16, tag="E", name=f"E{h}{p}{kt}")
                    for j, qc in enumerate(qcs):
                        ns = so if qc == qc0 else 0
                        nc.tensor.matmul(
                            S[:, 512 * j + ns : 512 * (j + 1)],
                            kT[:, 128 * kt : 128 * (kt + 1)],
                            qT[:, 512 * qc + ns : 512 * (qc + 1)],
                            start=True,
                            stop=True,
                        )
                    es = so if qcs[0] == qc0 else 0
                    ee = 512 * len(qcs)
                    nc.scalar.activation(
                        E[:, es:ee], S[:, es:ee], AF.Exp, scale=0.125
                    )
                    if qcs[0] == qc0:
                        nc.vector.tensor_mul(
                            E[:, so : so + 128], E[:, so : so + 128], mask_sb[:]
                        )
                    for j, qc in enumerate(qcs):
                        ns = so if qc == qc0 else 0
                        nc.tensor.matmul(
                            av[:, pqcs.index(qc), ns:512],
                            v_sb[kt][:, (D + 1) * h : (D + 1) * (h + 1)],
                            E[:, 512 * j + ns : 512 * (j + 1)],
                            start=(kt == 0),
                            stop=(kt == 4 * qc + 3),
                            skip_group_check=True,
                        )
                    if (p, kt) in ((0, 7), (1, 15)):
                        # normalize this pass: y = av[:D] / av[D]. Pack both
                        # denom rows into partitions {0,32} so one DVE
                        # reciprocal covers them (8 cyc/elem on the free dim;
                        # partition packing is the only parallelism), then
                        # bounce through DRAM for the partition-broadcast DMA.
                        pqcs_n = (0, 1) if p == 0 else (2, 3)
                        dn = bcp.tile([33, 512], F32, tag="dn", name=f"dn{h}{p}")
                        nc.vector.tensor_copy(dn[0:1, :], av[D : D + 1, 0, :])
                        nc.vector.tensor_copy(dn[32:33, :], av[D : D + 1, 1, :])
                        rc = bcp.tile([33, 512], F32, tag="rc", name=f"rc{h}{p}")
                        nc.vector.reciprocal_approx_fast(out=rc[:], in_=dn[:])
                        scr = dpool.tile([2, 512], F32, tag="scr", name=f"scr{h}{p}")
                        nc.sync.dma_start(scr[0:1, :], rc[0:1, :])
                        nc.sync.dma_start(scr[1:2, :], rc[32:33, :])
                        bc = bcp.tile([64, 1024], F32, tag="bc", name=f"bc{h}{p}")
                        bc_src = bass.AP(
                            tensor=scr[:].tensor,
                            offset=scr[:].offset,
                            ap=[[0, 64], [1, 1024]],
                        )
                        nc.sync.dma_start(bc[:], bc_src)
                        for j, qc in enumerate(pqcs_n):
                            nc.vector.tensor_mul(
                                yrow[:, 512 * qc : 512 * (qc + 1)],
                                av[0:D, j, :],
                                bc[:, 512 * j : 512 * (j + 1)],
                            )
            # ---- c_proj partial: out[o, t] += wpT^T @ yT  (local 256 channels) ----
            for tcn in (2, 3, 0, 1):
                for mt in range(C // 128):
                    po = ps.tile([128, 1024], F32, tag="S", name=f"po{mt}{tcn}")[:, 0:512]
                    for ky in range(CL // 128):
                        nc.tensor.matmul(
                            po[:],
                            wp_sb[ky][:, 128 * mt : 128 * (mt + 1)],
                            yT_sb[ky][:, 512 * tcn : 512 * (tcn + 1)],
                            start=(ky == 0),
                            stop=(ky == CL // 128 - 1),
                        )
                    ob = wp.tile([128, 512], BF16, tag="ob")
                    nc.vector.tensor_copy(ob[:], po[:])
                    eng = nc.gpsimd if (4 * mt + tcn) % 2 == 0 else nc.sync
                    eng.dma_start(
                        out_d[128 * mt : 128 * (mt + 1), 512 * tcn : 512 * (tcn + 1)],
                        ob[:],
                    )
            ps_cm.__exit__(None, None, None)
    nc.finalize()
    return nc


_GRAPH_CACHE = {}


def kernel(x, W_attn, b_attn, W_proj, b_proj, bV, **_unused):
    x = np.asarray(x, dtype=np.float32)
    W_attn = np.asarray(W_attn, dtype=np.float32)
    b_attn = np.asarray(b_attn, dtype=np.float32)
    W_proj = np.asarray(W_proj, dtype=np.float32)
    b_proj = np.asarray(b_proj, dtype=np.float32)
    bV = np.asarray(bV, dtype=np.float32)

    bf = ml_dtypes.bfloat16
    xT = [np.ascontiguousarray(x[b].T).astype(bf) for b in range(B)]
    mask = np.triu(np.ones((128, 128), np.float32)).astype(bf)

    in_maps = []
    for core in range(N_CORES):
        b, g = core // 4, core % 4
        rq = slice(CL * g, CL * (g + 1))
        rk = slice(C + CL * g, C + CL * (g + 1))
        rv = slice(2 * C + CL * g, 2 * C + CL * (g + 1))
        wqkT = np.ascontiguousarray(
            np.concatenate([W_attn[rq].T, W_attn[rk].T], axis=1)
        ).astype(bf)
        wvT = np.ascontiguousarray(W_attn[rv].T).astype(bf)
        wpT = np.ascontiguousarray(W_proj[:, CL * g : CL * (g + 1)].T).astype(bf)
        bqk = np.concatenate([b_attn[rq], b_attn[rk]]).reshape(4, 128).T
        bqk = np.ascontiguousarray(bqk).astype(np.float32)
        bv = (bV[HPC * g : HPC * (g + 1)].reshape(1, CL) + b_attn[rv][None]).astype(bf)
        in_maps.append(
            {
                "xT": xT[b],
                "wqkT": wqkT,
                "wvT": wvT,
                "wpT": wpT,
                "bqk": bqk,
                "bv": bv,
                "mask": mask,
            }
        )

    if "nc" not in _GRAPH_CACHE:
        _GRAPH_CACHE["nc"] = build_graph()
    nc = _GRAPH_CACHE["nc"]
    _GRAPH_CACHE["in_maps"] = in_maps

    res = run_bass_kernel_spmd(nc, in_maps, core_ids=list(range(N_CORES)))
    outs = [res.results[i]["out"] for i in range(N_CORES)]  # [C, T] fp32 partials

    out = np.empty((B, T, C), dtype=np.float32)
    for b in range(B):
        acc = outs[4 * b].astype(np.float32)
        for g in range(1, 4):
            acc += outs[4 * b + g].astype(np.float32)
        out[b] = acc.T + b_proj[None, :]
    return out

